# revision 55
# baseline (speedup 1.0000x reference)
"""Trainium2 Bass kernel for DepthwiseXCorrAug.

Computes, for B=64 samples sharded 8-per-core across 8 NeuronCores:
  k = relu(bn(conv3x3_valid(kernel_in, w_k)))     # [B,256,5,5]
  s = relu(bn(conv3x3_same(search_in, w_s)))      # [B,256,31,31]
  out = per-sample per-channel xcorr(s, k), pad 2 # [B,256,31,31]

Device strategy (per core):
  - everything in bf16 on the PE (weights, activations); accumulate f32 PSUM.
    (bf16 full-width matmuls run at exactly N/2.4GHz with no weight-swap
    bubble; fp32r pays +77ns/MM. fp8 gains nothing and fails numerics.)
  - conv branches as full-width (ci-block x 3x3-tap) matmuls accumulated in
    PSUM; BN folded into weights on host, bias+ReLU by ScalarE on eviction.
  - depthwise xcorr as bf16 64x64-diagonal-weight tile matmuls
    (tile_position): per (sample-pair, ob) chunk, 4 tiles x 25 taps
    accumulate in 4 PSUM banks. This path is moving-stream-bus bound
    (~240 elem/cycle in tiled mode).
  - conv_k first (smallest DMA deps -> PE starts early), conv_s pairs and
    xcorr chunks interleaved; outputs stream out as bf16 full-row DMAs
    spread across Sync/GpSimd queues; host converts to f32.
"""

import sys

sys.path.insert(0, "/opt/trn_rl_repo")

import numpy as np

import concourse.bass as bass
import concourse.mybir as mybir
import concourse.tile as tile
from concourse import bacc
from concourse.bass_utils import run_bass_kernel_spmd

EPS = 1e-5
N_CORES = 8
B, CIN, HID = 64, 256, 256
SPC = B // N_CORES  # samples per core

_cached_nc = None
last_results = None  # set by kernel(); used by test harness for profiling


def _build_program():
    f32 = mybir.dt.float32
    bf16 = mybir.dt.bfloat16
    RELU = mybir.ActivationFunctionType.Relu

    nc = bacc.Bacc("TRN2", target_bir_lowering=False, debug=False,
                   num_devices=N_CORES)

    wTs_d = [nc.dram_tensor(f"wTs{cb}", [128, 2304], bf16, kind="ExternalInput").ap()
             for cb in range(2)]
    wTk_d = [nc.dram_tensor(f"wTk{cb}", [128, 2304], bf16, kind="ExternalInput").ap()
             for cb in range(2)]
    xk_d = [nc.dram_tensor(f"xk{cb}", [128, 1800], bf16, kind="ExternalInput").ap()
            for cb in range(2)]
    xs_d = nc.dram_tensor("xs", [SPC, 128, 2 * 33 * 34], bf16, kind="ExternalInput").ap()
    bk_d = nc.dram_tensor("bk", [2, 128, 1], f32, kind="ExternalInput").ap()
    bs_d = nc.dram_tensor("bs", [2, 128, 1], f32, kind="ExternalInput").ap()
    m64rep_d = nc.dram_tensor("m64rep", [128, 1600], bf16, kind="ExternalInput").ap()
    out_d = nc.dram_tensor("out", [SPC, CIN, 31, 31], bf16, kind="ExternalOutput").ap()
    out_flat = out_d.rearrange("s c h w -> s c (h w)")

    with tile.TileContext(nc) as tc:
        with tc.tile_pool(name="wp", bufs=1) as wp, \
             tc.tile_pool(name="spin", bufs=8) as spin_pool, \
             tc.tile_pool(name="spoutp", bufs=1) as spout_pool, \
             tc.tile_pool(name="stripp", bufs=1) as strip_pool, \
             tc.tile_pool(name="xop", bufs=8) as xout_pool, \
             tc.tile_pool(name="ps", bufs=8, space="PSUM") as ps:

            # ---- persistent inputs (weights split per (cb, ob) for precise
            # DMA dependencies) ----
            wTs = {(cb, ob): wp.tile([128, 1152], bf16, tag=f"wTs{cb}{ob}",
                                     name=f"wTs{cb}{ob}")
                   for cb in range(2) for ob in range(2)}
            wTk = {(cb, ob): wp.tile([128, 1152], bf16, tag=f"wTk{cb}{ob}",
                                     name=f"wTk{cb}{ob}")
                   for cb in range(2) for ob in range(2)}
            xk = [wp.tile([128, 1800], bf16, tag=f"xk{cb}", name=f"xk{cb}")
                  for cb in range(2)]
            bk = [wp.tile([128, 1], f32, tag=f"bk{ob}", name=f"bk{ob}")
                  for ob in range(2)]
            bs = [wp.tile([128, 1], f32, tag=f"bs{ob}", name=f"bs{ob}")
                  for ob in range(2)]
            m64rep = wp.tile([128, 1600], bf16, tag="m64rep", name="m64rep")
            kf = [wp.tile([128, 200], f32, tag=f"kf{ob}", name=f"kf{ob}")
                  for ob in range(2)]

            # ---- spout tiles (bf16); zero only the 2-wide borders ----
            # 8 physical tiles, reused by samples s and s+4 (deps tracked)
            spout = {}
            for s in range(4):
                for ob in range(2):
                    sp = spout_pool.tile([128, 35 * 35], bf16,
                                         tag=f"spout{s}_{ob}", name=f"spout{s}_{ob}")
                    spout[(s, ob)] = sp
                    spout[(s + 4, ob)] = sp
                    eng = nc.vector if (s + ob) % 2 == 0 else nc.gpsimd
                    # zero the 2-wide border frame; interior is written by
                    # the conv_s activation
                    v = sp[:].rearrange("p (r c) -> p r c", r=35, c=35)
                    eng.memset(sp[:, 0:70], 0.0)
                    eng.memset(sp[:, 1155:1225], 0.0)
                    eng.memset(v[:, 2:33, 0:2], 0.0)
                    eng.memset(v[:, 2:33, 33:35], 0.0)

            # spin prefetch state
            spin_views = {}

            def prefetch_pair(pair, deng):
                s0 = pair * 2
                for s in (s0, s0 + 1):
                    t_in = spin_pool.tile([128, 2 * 33 * 34], bf16,
                                          tag="spin", name=f"spin{s}")
                    deng.dma_start(t_in[:], xs_d[s])
                    for cb in range(2):
                        spin_views[(s, cb)] = t_in[
                            :, cb * 1122:(cb + 1) * 1122].rearrange(
                            "p (h w) -> p h w", h=33, w=34)

            # ---- DMA order: conv_k deps first so PE starts ASAP; issue
            # spread across engines (sync: conv_k path, vector: conv_s path,
            # scalar: small tensors) ----
            for cb in range(2):
                for c0 in (0, 576):
                    nc.sync.dma_start(wTk[(cb, 0)][:, c0:c0 + 576],
                                      wTk_d[cb][:, c0:c0 + 576])
                for c0 in (0, 600, 1200):
                    nc.sync.dma_start(xk[cb][:, c0:c0 + 600],
                                      xk_d[cb][:, c0:c0 + 600])
            prefetch_pair(0, nc.sync)
            for cb in range(2):
                nc.sync.dma_start(wTk[(cb, 1)][:], wTk_d[cb][:, 1152:2304])
            for cb in range(2):
                nc.sync.dma_start(wTs[(cb, 0)][:], wTs_d[cb][:, 0:1152])
            for ob in range(2):
                nc.scalar.dma_start(bk[ob][:], bk_d[ob])
                nc.scalar.dma_start(bs[ob][:], bs_d[ob])
            nc.scalar.dma_start(m64rep[:], m64rep_d)
            for cb in range(2):
                nc.scalar.dma_start(wTs[(cb, 1)][:], wTs_d[cb][:, 1152:2304])
            for p in (1, 2, 3):
                prefetch_pair(p, nc.scalar)

            # ---- conv_k: all 8 samples batched on the free dim (N=256) ----
            def emit_conv_k():
                for ob in range(2):
                    pk = ps.tile([128, 512], f32, tag="mm", name=f"pk{ob}")
                    idx = 0
                    for cb in range(2):
                        for t in range(9):
                            nc.tensor.matmul(
                                pk[:, 0:200],
                                wTk[(cb, ob)][:, t * 128:(t + 1) * 128],
                                xk[cb][:, t * 200:(t + 1) * 200],
                                start=(idx == 0), stop=(idx == 17))
                            idx += 1
                    nc.scalar.activation(kf[ob][:], pk[:, 0:200], RELU,
                                         bias=bk[ob][:, 0:1], scale=1.0)

            # ---- strips: bf16 64-diag weights, one DVE op per (s, ob) ----
            # 8 physical tiles, reused by samples s and s+4
            strips = {}
            for s in range(4):
                for ob in range(2):
                    st = strip_pool.tile(
                        [128, 1600], bf16,
                        tag=f"strip{s}_{ob}", name=f"strip{s}_{ob}")
                    strips[(s, ob)] = st
                    strips[(s + 4, ob)] = st

            def emit_strips(samples, skip=frozenset()):
                for ob in range(2):
                    for s in samples:
                        if (s, ob) in skip:
                            continue
                        kfb = kf[ob][:, s * 25:(s + 1) * 25].unsqueeze(
                            -1).broadcast_to([128, 25, 64])
                        nc.vector.tensor_tensor(
                            strips[(s, ob)][:], m64rep[:], kfb,
                            mybir.AluOpType.mult)

            # ---- conv_s: one pair of samples, both ob blocks ----
            def conv_s_pair(pair):
                s0 = pair * 2
                views = spin_views
                for ob in range(2):
                    ptiles = {}
                    for s in (s0, s0 + 1):
                        for ci in range(2):
                            ptiles[(s, ci)] = ps.tile(
                                [128, 512], f32, tag="mm",
                                name=f"pc{s}_{ob}_{ci}")
                    idx = 0
                    for cb in range(2):
                        for t in range(9):
                            dy, dx = t // 3, t % 3
                            lhsT = wTs[(cb, ob)][:, t * 128:(t + 1) * 128]
                            for s in (s0, s0 + 1):
                                for ci, (y0, nr) in enumerate([(0, 16), (16, 15)]):
                                    nc.tensor.matmul(
                                        ptiles[(s, ci)][:, 0:nr * 31],
                                        lhsT,
                                        views[(s, cb)][:, y0 + dy:y0 + dy + nr,
                                                       dx:dx + 31],
                                        start=(idx == 0), stop=(idx == 17))
                            idx += 1
                    for s in (s0, s0 + 1):
                        sov = spout[(s, ob)][:].rearrange(
                            "p (h w) -> p h w", h=35, w=35)
                        for ci, (y0, nr) in enumerate([(0, 16), (16, 15)]):
                            pv = ptiles[(s, ci)][:, 0:nr * 31].rearrange(
                                "p (h w) -> p h w", h=nr, w=31)
                            nc.scalar.activation(
                                sov[:, 2 + y0:2 + y0 + nr, 2:33],
                                pv[:, :, :], RELU,
                                bias=bs[ob][:, 0:1], scale=1.0)

            # ---- xcorr: 64x64-tile chunk per (q, ob): sample pair (2q, 2q+1)
            CI_SPEC = [(0, 16), (16, 15)]

            def xcorr_chunk(q, ob):
                sovs = [spout[(q * 2 + j, ob)][:].rearrange(
                    "p (h w) -> p h w", h=35, w=35) for j in range(2)]
                px = {}
                for ci in range(2):
                    for i in range(2):
                        px[(ci, i)] = ps.tile([128, 512], f32, tag="mm",
                                              name=f"px{q}_{ob}_{ci}_{i}")
                for t in range(25):
                    dy, dx = t // 5, t % 5
                    for i in range(2):
                        for j in range(2):
                            st = strips[(q * 2 + j, ob)]
                            lhsT = st[64 * i:64 * i + 64, t * 64:(t + 1) * 64]
                            for ci, (y0, nr) in enumerate(CI_SPEC):
                                nc.tensor.matmul(
                                    px[(ci, i)][64 * j:64 * j + 64, 0:nr * 31],
                                    lhsT,
                                    sovs[j][64 * i:64 * i + 64,
                                            y0 + dy:y0 + dy + nr, dx:dx + 31],
                                    start=(t == 0), stop=(t == 24),
                                    tile_position=(64 * i, 64 * j))
                n_ev = 0
                for i in range(2):
                    xo = xout_pool.tile([128, 1024], bf16, tag="xo",
                                        name=f"xo{q}_{ob}_{i}")
                    for ci, (y0, nr) in enumerate(CI_SPEC):
                        N = nr * 31
                        if n_ev % 2 == 0:
                            nc.vector.tensor_copy(
                                xo[:, y0 * 31:y0 * 31 + N], px[(ci, i)][:, 0:N])
                        else:
                            nc.scalar.copy(
                                xo[:, y0 * 31:y0 * 31 + N], px[(ci, i)][:, 0:N])
                        n_ev += 1
                    dst = out_flat[q * 2:q * 2 + 2,
                                   ob * 128 + 64 * i:ob * 128 + 64 * i + 64,
                                   0:961]
                    deng = nc.gpsimd if (q + ob + i) % 2 == 0 else nc.sync
                    deng.dma_start(dst, xo[:, 0:961])

            emit_conv_k()
            emit_strips(range(4))
            conv_s_pair(0)
            conv_s_pair(1)
            xcorr_chunk(0, 0)
            xcorr_chunk(0, 1)
            xcorr_chunk(1, 0)
            xcorr_chunk(1, 1)
            emit_strips(range(4, SPC))
            conv_s_pair(2)
            conv_s_pair(3)
            for q in (2, 3):
                xcorr_chunk(q, 0)
                xcorr_chunk(q, 1)

    nc.compile()
    return nc


def _host_prep(kernel, search, w_k, g_k, b_k, m_k, v_k, w_s, g_s, b_s, m_s, v_s):
    import ml_dtypes
    bf16 = ml_dtypes.bfloat16

    def fold(w, g, b, m, v):
        scale = g / np.sqrt(v + EPS)
        return (w * scale[:, None, None, None]).astype(np.float32), \
               (b - m * scale).astype(np.float32)

    wkf, bias_k = fold(w_k, g_k, b_k, m_k, v_k)
    wsf, bias_s = fold(w_s, g_s, b_s, m_s, v_s)

    def packT(w):  # [o, ci, 3, 3] -> [cb, ci, (ob, t, o)] bf16
        arr = w.reshape(2, 128, 2, 128, 9).transpose(2, 3, 0, 4, 1)
        return np.ascontiguousarray(arr, dtype=np.float32).astype(
            bf16).reshape(2, 128, 2304)

    wTk = packT(wkf)
    wTs = packT(wsf)

    M64 = np.zeros((128, 64), dtype=np.float32)
    for p in range(128):
        M64[p, p % 64] = 1.0
    M64REP = np.tile(M64, (1, 25)).astype(bf16)

    bk = np.ascontiguousarray(bias_k.reshape(2, 128, 1))
    bs = np.ascontiguousarray(bias_s.reshape(2, 128, 1))

    in_maps = []
    for core in range(N_CORES):
        kin = kernel[core * SPC:(core + 1) * SPC]
        sin = search[core * SPC:(core + 1) * SPC]

        Xk = np.zeros((2, 128, 9, 200), dtype=np.float32)
        for t in range(9):
            dy, dx = t // 3, t % 3
            p = kin[:, :, dy:dy + 5, dx:dx + 5].reshape(SPC, 2, 128, 25)
            Xk[:, :, t, :] = p.transpose(1, 2, 0, 3).reshape(2, 128, 200)
        Xk = Xk.astype(bf16).reshape(2, 128, 1800)

        Xs = np.zeros((SPC, 2, 128, 33, 34), dtype=np.float32)
        Xs[:, :, :, 1:32, 1:32] = sin.reshape(SPC, 2, 128, 31, 31)
        Xs = np.ascontiguousarray(
            Xs.transpose(0, 2, 1, 3, 4)).astype(bf16).reshape(
            SPC, 128, 2 * 33 * 34)

        in_maps.append({
            "wTs0": wTs[0], "wTs1": wTs[1],
            "wTk0": wTk[0], "wTk1": wTk[1],
            "xk0": Xk[0], "xk1": Xk[1],
            "xs": Xs, "bk": bk, "bs": bs, "m64rep": M64REP,
        })
    return in_maps


def kernel(kernel, search, w_k, g_k, b_k, m_k, v_k, w_s, g_s, b_s, m_s, v_s,
           _trace=False):
    global _cached_nc, last_results
    args = [np.ascontiguousarray(np.asarray(x, dtype=np.float32)) for x in
            (kernel, search, w_k, g_k, b_k, m_k, v_k, w_s, g_s, b_s, m_s, v_s)]
    if _cached_nc is None:
        _cached_nc = _build_program()
    nc = _cached_nc
    in_maps = _host_prep(*args)
    res = run_bass_kernel_spmd(nc, in_maps, core_ids=list(range(N_CORES)),
                               trace=_trace)
    last_results = res
    out = np.concatenate([res.results[i]["out"] for i in range(N_CORES)], axis=0)
    return np.ascontiguousarray(out.astype(np.float32))
